# revision 3
# baseline (speedup 1.0000x reference)
"""Trainium2 Bass kernel for the Mahalanobis-softmax loss + H matrix.

Computes, for X[16384,128], T[16384], means[2048,128], log_vars[2048,128]:
  lv   = clip(log_vars, 0, 6);  ic = exp(-lv)
  Xn   = l2norm(X); mu = l2norm(means)
  D    = ||Xn - mu||^2_ic  (Mahalanobis, expanded to GEMMs)
  loss = mean_n( -log_softmax(-TAU*D)[n, T[n]] )
  H    = onehot(T) + exp(-ALPHA*D) * (1 - onehot(T))

Sharding: data-parallel over N across 8 NeuronCores (X/T sharded,
means/log_vars replicated); H shards concatenated and per-core partial
loss sums reduced on host.

Device-side formulation (per core, S := -TAU*D computed straight in PSUM):
  S[n,c] = st1(:,n).e(:,c) + st2(:,n).ast(:,c) + bs(c)      (3 matmuls/chunk)
    st1[f,n] = -TAU * rn[n]^2 * X[n,f]^2   (stationary 1)
    st2[f,n] = rn[n] * X[n,f] = Xn^T       (stationary 2)
    e[f,c]   = exp(-clip(lv,0,6))[c,f] = ic^T     (moving 1)
    ast[f,c] = 2*TAU*mu[c,f]*ic[c,f]              (moving 2)
    bs[c]    = -TAU * sum_f mu^2*ic               (K=1 bias row)
  ACT: sexp[n] = sum_c exp(S) (free accum), htile = exp((ALPHA/TAU)*S)
  DVE: hout = max(iota==T[n], htile)  -> final H tile (label entry == 1.0)
       accum ah[n] = sum_c hout  => Hexp_T = hsum + 1 - ah (label term)
  loss_n = ln(sexp_n) - (TAU/ALPHA)*ln(Hexp_T[n])
"""

import os
import sys

import numpy as np

for _p in ("/opt/trn_rl_repo", "/root/.axon_site/_ro/trn_rl_repo"):
    if _p not in sys.path and os.path.isdir(_p):
        sys.path.insert(0, _p)

N = 16384
C = 2048
F = 128
TAU = 32.0
ALPHA = 0.9
Q = ALPHA / TAU          # scale turning S=-TAU*D into -ALPHA*D
NCORES = 8
NS = N // NCORES         # rows per core (2048)
NT = NS // 128           # 128-row tiles per core (16)

_MODULE_CACHE = {}


def _build_module():
    if "nc" in _MODULE_CACHE:
        return _MODULE_CACHE["nc"]

    import concourse.bacc as bacc
    import concourse.bass as bass
    import concourse.tile as tile
    from concourse import mybir

    dt = mybir.dt
    f32 = dt.float32
    Alu = mybir.AluOpType
    Act = mybir.ActivationFunctionType

    nc = bacc.Bacc(
        "TRN2",
        target_bir_lowering=False,
        debug=False,
        enable_asserts=False,
        num_devices=NCORES,
    )

    # I/O (per core)
    xt_d = nc.dram_tensor("xt", [F, NS], f32, kind="ExternalInput")       # X.T shard
    mt_d = nc.dram_tensor("mt", [F, C], f32, kind="ExternalInput")        # means.T
    lvt_d = nc.dram_tensor("lvt", [F, C], f32, kind="ExternalInput")      # log_vars.T
    tf_d = nc.dram_tensor("tf", [128, NT], f32, kind="ExternalInput")     # labels, [p, t]
    iota_d = nc.dram_tensor("iota_row", [C], f32, kind="ExternalInput")   # arange(C)
    h_d = nc.dram_tensor("h", [NS, C], f32, kind="ExternalOutput")
    loss_d = nc.dram_tensor("loss", [128, 1], f32, kind="ExternalOutput")
    scr_d = nc.dram_tensor("scr", [8, 2048], f32)                         # bounce scratch

    def bcast_row_ap(row_ap, parts=128):
        # DRAM row [K] -> [parts, K] with partition step 0 (DMA broadcast read)
        return bass.AP(
            tensor=row_ap.tensor, offset=row_ap.offset, ap=[[0, parts]] + list(row_ap.ap)
        )

    with tile.TileContext(nc) as tc:
        from contextlib import ExitStack

        ctx = ExitStack()
        with ctx:
            singles = ctx.enter_context(tc.tile_pool(name="singles", bufs=1))
            tiny = ctx.enter_context(tc.tile_pool(name="tiny", bufs=1))

            # ---------------- prep phase ----------------
            # persistent big tiles
            e_t = singles.tile([128, C], f32)      # ic^T
            ast_t = singles.tile([128, C], f32)    # 2*TAU*mu*ic (^T)
            st1_t = singles.tile([128, NS], f32)   # -TAU*rn^2*X^2 (^T)
            st2_t = singles.tile([128, NS], f32)   # Xn^T
            iot_t = singles.tile([128, C], f32)    # iota bcast
            bs_t = singles.tile([1, C], f32)       # -TAU*b row
            tf_t = singles.tile([128, NT], f32)
            ones_col = tiny.tile([128, 1], f32)
            ones_row = tiny.tile([1, 128], f32)
            sexp_t = singles.tile([128, NT], f32)
            hsum_t = singles.tile([128, NT], f32)
            ah_t = singles.tile([128, NT], f32)

            nc.vector.memset(ones_col[:], 1.0)
            nc.vector.memset(ones_row[:], 1.0)

            nc.sync.dma_start(out=tf_t[:], in_=tf_d[:, :])
            nc.gpsimd.dma_start(out=iot_t[:], in_=bcast_row_ap(iota_d[:]))

            with tc.tile_pool(name="prep", bufs=1) as prep, \
                 tc.tile_pool(name="prep_psum", bufs=2, space="PSUM") as ppsum:
                xt_t = prep.tile([128, NS], f32)
                mt_t = prep.tile([128, C], f32)
                lvt_t = prep.tile([128, C], f32)
                xsq_t = prep.tile([128, NS], f32)
                msq_t = prep.tile([128, C], f32)
                p1_t = prep.tile([128, C], f32)
                bmat_t = prep.tile([128, C], f32)

                nc.sync.dma_start(out=xt_t[:], in_=xt_d[:, :])
                nc.sync.dma_start(out=mt_t[:], in_=mt_d[:, :])
                nc.sync.dma_start(out=lvt_t[:], in_=lvt_d[:, :])

                # ic^T = exp(-clip(lv,0,6)); clip fused in one DVE op (in-place)
                nc.vector.tensor_scalar(
                    out=lvt_t[:], in0=lvt_t[:], scalar1=0.0, scalar2=6.0,
                    op0=Alu.max, op1=Alu.min,
                )
                nc.scalar.activation(out=e_t[:], in_=lvt_t[:], func=Act.Exp, scale=-1.0)

                # squares
                nc.vector.tensor_mul(xsq_t[:], xt_t[:], xt_t[:])
                nc.vector.tensor_mul(msq_t[:], mt_t[:], mt_t[:])

                # column norms^2 via ones-matmul -> [1, 2048] rows in PSUM
                nx_ps = ppsum.tile([128, NS], f32, tag="ppsum")
                nm_ps = ppsum.tile([128, C], f32, tag="ppsum")
                for ch in range(4):
                    cs = slice(ch * 512, (ch + 1) * 512)
                    nc.tensor.matmul(
                        out=nx_ps[0:1, cs], lhsT=ones_col[:], rhs=xsq_t[:, cs],
                        start=True, stop=True,
                    )
                    nc.tensor.matmul(
                        out=nm_ps[0:1, cs], lhsT=ones_col[:], rhs=msq_t[:, cs],
                        start=True, stop=True,
                    )
                # evacuate PSUM rows to SBUF (DMA cannot read PSUM), then bounce
                nx_row = tiny.tile([1, NS], f32)
                nm_row = tiny.tile([1, C], f32)
                nc.vector.tensor_copy(nx_row[:], nx_ps[0:1, :])
                nc.vector.tensor_copy(nm_row[:], nm_ps[0:1, :])
                nc.sync.dma_start(out=scr_d[0:1, :], in_=nx_row[:])
                nc.sync.dma_start(out=scr_d[1:2, :], in_=nm_row[:])

                nx2_t = tiny.tile([128, 16], f32)
                nm2_t = tiny.tile([128, 16], f32)
                nc.sync.dma_start(
                    out=nx2_t[:], in_=scr_d[0, :].rearrange("(p j) -> p j", p=128)
                )
                nc.sync.dma_start(
                    out=nm2_t[:], in_=scr_d[1, :].rearrange("(p j) -> p j", p=128)
                )

                # rsqrt = exp(-0.5*ln(x)) (keeps everything in the exp/ln table set)
                # guard against zero-norm rows like the reference's max(norm, 1e-12)
                nc.vector.tensor_scalar(
                    out=nx2_t[:], in0=nx2_t[:], scalar1=1e-24, scalar2=None, op0=Alu.max
                )
                nc.vector.tensor_scalar(
                    out=nm2_t[:], in0=nm2_t[:], scalar1=1e-24, scalar2=None, op0=Alu.max
                )
                lnx_t = tiny.tile([128, 16], f32)
                lnm_t = tiny.tile([128, 16], f32)
                nc.scalar.activation(out=lnx_t[:], in_=nx2_t[:], func=Act.Ln)
                nc.scalar.activation(out=lnm_t[:], in_=nm2_t[:], func=Act.Ln)
                rsx_t = tiny.tile([128, 16], f32)   # 1/||x||
                rsm_t = tiny.tile([128, 16], f32)   # 1/||mu||
                nc.scalar.activation(out=rsx_t[:], in_=lnx_t[:], func=Act.Exp, scale=-0.5)
                nc.scalar.activation(out=rsm_t[:], in_=lnm_t[:], func=Act.Exp, scale=-0.5)

                # derived per-column scale rows
                r2x_t = tiny.tile([128, 16], f32)   # -TAU * rn^2
                r3m_t = tiny.tile([128, 16], f32)   # 2*TAU*rsm
                r4m_t = tiny.tile([128, 16], f32)   # -TAU*rsm^2
                nc.vector.tensor_mul(r2x_t[:], rsx_t[:], rsx_t[:])
                nc.vector.tensor_scalar(
                    out=r2x_t[:], in0=r2x_t[:], scalar1=-TAU, scalar2=None, op0=Alu.mult
                )
                nc.vector.tensor_scalar(
                    out=r3m_t[:], in0=rsm_t[:], scalar1=2.0 * TAU, scalar2=None, op0=Alu.mult
                )
                nc.vector.tensor_mul(r4m_t[:], rsm_t[:], rsm_t[:])
                nc.vector.tensor_scalar(
                    out=r4m_t[:], in0=r4m_t[:], scalar1=-TAU, scalar2=None, op0=Alu.mult
                )

                # bounce scale rows out and broadcast back to [128, 2048]
                nc.sync.dma_start(
                    out=scr_d[4, :].rearrange("(p j) -> p j", p=128), in_=rsx_t[:]
                )
                nc.sync.dma_start(
                    out=scr_d[5, :].rearrange("(p j) -> p j", p=128), in_=r2x_t[:]
                )
                nc.sync.dma_start(
                    out=scr_d[6, :].rearrange("(p j) -> p j", p=128), in_=r3m_t[:]
                )
                nc.sync.dma_start(
                    out=scr_d[7, :].rearrange("(p j) -> p j", p=128), in_=r4m_t[:]
                )
                rxB_t = prep.tile([128, NS], f32)
                r2xB_t = prep.tile([128, NS], f32)
                r3mB_t = prep.tile([128, C], f32)
                r4mB_t = prep.tile([128, C], f32)
                nc.gpsimd.dma_start(out=rxB_t[:], in_=bcast_row_ap(scr_d[4, :]))
                nc.gpsimd.dma_start(out=r2xB_t[:], in_=bcast_row_ap(scr_d[5, :]))
                nc.gpsimd.dma_start(out=r3mB_t[:], in_=bcast_row_ap(scr_d[6, :]))
                nc.gpsimd.dma_start(out=r4mB_t[:], in_=bcast_row_ap(scr_d[7, :]))

                # stationaries (DVE) and class moving tensors (gpsimd)
                nc.vector.tensor_mul(st1_t[:], xsq_t[:], r2xB_t[:])
                nc.vector.tensor_mul(st2_t[:], xt_t[:], rxB_t[:])
                nc.gpsimd.tensor_mul(p1_t[:], mt_t[:], e_t[:])
                nc.gpsimd.tensor_mul(ast_t[:], p1_t[:], r3mB_t[:])
                nc.gpsimd.tensor_mul(msq_t[:], msq_t[:], e_t[:])
                nc.gpsimd.tensor_mul(bmat_t[:], msq_t[:], r4mB_t[:])

                # bias row: bs[c] = colsum(bmat)
                bs_ps = ppsum.tile([128, C], f32, tag="ppsum")
                for ch in range(4):
                    cs = slice(ch * 512, (ch + 1) * 512)
                    nc.tensor.matmul(
                        out=bs_ps[0:1, cs], lhsT=ones_col[:], rhs=bmat_t[:, cs],
                        start=True, stop=True,
                    )
                nc.vector.tensor_copy(bs_t[:], bs_ps[0:1, :])

            # ---------------- main loop ----------------
            with tc.tile_pool(name="spsum", bufs=2, space="PSUM") as spsum, \
                 tc.tile_pool(name="hpool", bufs=3) as hpool, \
                 tc.tile_pool(name="hopool", bufs=3) as hopool, \
                 tc.tile_pool(name="pscr", bufs=1) as pscrp:
                pscr_t = pscrp.tile([128, C], f32)  # exp(S) full-size sink
                for t in range(NT):
                    ns = slice(t * 128, (t + 1) * 128)
                    s_ps = spsum.tile([128, C], f32, tag="S")
                    for ch in range(4):
                        cs = slice(ch * 512, (ch + 1) * 512)
                        nc.tensor.matmul(
                            out=s_ps[:, cs], lhsT=st1_t[:, ns], rhs=e_t[:, cs],
                            start=True, stop=False,
                        )
                        nc.tensor.matmul(
                            out=s_ps[:, cs], lhsT=st2_t[:, ns], rhs=ast_t[:, cs],
                            start=False, stop=False,
                        )
                        nc.tensor.matmul(
                            out=s_ps[:, cs], lhsT=ones_row[:], rhs=bs_t[:, cs],
                            start=False, stop=True,
                        )
                    # softmax sum (accum is free on the ACT pass)
                    nc.scalar.activation(
                        out=pscr_t[:], in_=s_ps[:], func=Act.Exp,
                        accum_out=sexp_t[:, t : t + 1],
                    )
                    # H tile = exp((ALPHA/TAU)*S), accum for label-term recovery
                    ht = hpool.tile([128, C], f32, tag="H")
                    nc.scalar.activation(
                        out=ht[:], in_=s_ps[:], func=Act.Exp, scale=Q,
                        accum_out=hsum_t[:, t : t + 1],
                    )
                    # blend one-hot: hout = max(iota == T[n], htile); accum -> ah
                    ho = hopool.tile([128, C], f32, tag="HO")
                    nc.vector.scalar_tensor_tensor(
                        out=ho[:], in0=iot_t[:], scalar=tf_t[:, t : t + 1],
                        in1=ht[:], op0=Alu.is_equal, op1=Alu.max,
                        accum_out=ah_t[:, t : t + 1],
                    )
                    nc.sync.dma_start(out=h_d[ns, :], in_=ho[:])

            # ---------------- loss epilogue ----------------
            hexpt_t = tiny.tile([128, NT], f32)
            lnh_t = tiny.tile([128, NT], f32)
            lns_t = tiny.tile([128, NT], f32)
            lossv_t = tiny.tile([128, NT], f32)
            loss_t = tiny.tile([128, 1], f32)
            # Hexp_T = hsum + 1 - ah
            nc.vector.scalar_tensor_tensor(
                out=hexpt_t[:], in0=hsum_t[:], scalar=1.0, in1=ah_t[:],
                op0=Alu.add, op1=Alu.subtract,
            )
            # guard: exact-1.0 label entries can make hsum+1-ah round to <=0
            nc.vector.tensor_scalar(
                out=hexpt_t[:], in0=hexpt_t[:], scalar1=1e-30, scalar2=None, op0=Alu.max
            )
            nc.scalar.activation(out=lnh_t[:], in_=hexpt_t[:], func=Act.Ln)
            nc.scalar.activation(out=lns_t[:], in_=sexp_t[:], func=Act.Ln)
            # loss_n = lns - (TAU/ALPHA)*lnh
            nc.vector.scalar_tensor_tensor(
                out=lossv_t[:], in0=lnh_t[:], scalar=-(TAU / ALPHA), in1=lns_t[:],
                op0=Alu.mult, op1=Alu.add,
            )
            nc.vector.reduce_sum(
                out=loss_t[:], in_=lossv_t[:], axis=mybir.AxisListType.X
            )
            nc.sync.dma_start(out=loss_d[:, :], in_=loss_t[:])

    nc.compile()
    _MODULE_CACHE["nc"] = nc
    return nc


def _make_in_maps(X, T, means, log_vars):
    X = np.asarray(X, dtype=np.float32)
    T = np.asarray(T)
    means = np.asarray(means, dtype=np.float32)
    log_vars = np.asarray(log_vars, dtype=np.float32)

    xt_full = np.ascontiguousarray(X.T)                     # [F, N]
    mt = np.ascontiguousarray(means.T)                      # [F, C]
    lvt = np.ascontiguousarray(log_vars.T)                  # [F, C]
    iota_row = np.arange(C, dtype=np.float32)

    in_maps = []
    for c in range(NCORES):
        sl = slice(c * NS, (c + 1) * NS)
        tf = np.ascontiguousarray(
            T[sl].astype(np.float32).reshape(NT, 128).T
        )                                                    # [128, NT]
        in_maps.append(
            {
                "xt": np.ascontiguousarray(xt_full[:, sl]),
                "mt": mt,
                "lvt": lvt,
                "tf": tf,
                "iota_row": iota_row,
            }
        )
    return in_maps


def _postprocess(results):
    h_parts = [results[c]["h"] for c in range(NCORES)]
    H = np.concatenate(h_parts, axis=0)
    loss_sum = 0.0
    for c in range(NCORES):
        loss_sum += float(np.sum(results[c]["loss"].astype(np.float64)))
    loss_mean = np.float32(loss_sum / N)
    return loss_mean, H


def kernel(X, T, means, log_vars):
    from concourse.bass_utils import run_bass_kernel_spmd

    nc = _build_module()
    in_maps = _make_in_maps(X, T, means, log_vars)
    res = run_bass_kernel_spmd(nc, in_maps, list(range(NCORES)))
    return _postprocess(res.results)


def run_sim(X, T, means, log_vars, core=0):
    """CoreSim single-core run for correctness debugging (no hardware)."""
    from concourse.bass_interp import CoreSim

    nc = _build_module()
    in_maps = _make_in_maps(X, T, means, log_vars)
    sim = CoreSim(nc, trace=False)
    for k, v in in_maps[core].items():
        sim.tensor(k)[:] = v
    sim.simulate()
    return {k: np.array(sim.tensor(k)) for k in ("h", "loss")}


# revision 6
# speedup vs baseline: 1.5629x; 1.5629x over previous
"""Trainium2 Bass kernel for the Mahalanobis-softmax loss + H matrix.

Computes, for X[16384,128], T[16384], means[2048,128], log_vars[2048,128]:
  lv   = clip(log_vars, 0, 6);  ic = exp(-lv)
  Xn   = l2norm(X); mu = l2norm(means)
  D    = ||Xn - mu||^2_ic  (Mahalanobis, expanded to GEMMs)
  loss = mean_n( -log_softmax(-TAU*D)[n, T[n]] )
  H    = onehot(T) + exp(-ALPHA*D) * (1 - onehot(T))

Sharding: data-parallel over N across 8 NeuronCores (X/T sharded,
means/log_vars replicated); H shards concatenated and per-core partial
loss sums reduced on host.

Device-side formulation (per core, S := -TAU*D computed straight in PSUM):
  S[n,c] = st1(:,n).e(:,c) + st2(:,n).ast(:,c) + bs(c)      (3 matmuls/chunk)
    st1[f,n] = -TAU * rn[n]^2 * X[n,f]^2   (stationary 1)
    st2[f,n] = rn[n] * X[n,f] = Xn^T       (stationary 2)
    e[f,c]   = exp(-clip(lv,0,6))[c,f] = ic^T     (moving 1)
    ast[f,c] = 2*TAU*mu[c,f]*ic[c,f]              (moving 2)
    bs[c]    = -TAU * sum_f mu^2*ic               (K=1 bias row)
  Main-loop matmul operands are float32r (1 cyc/row vs 4 for float32);
  they are all produced by DVE ops which round on write. Small prep
  colsums stay plain fp32 matmuls. Row->tile broadcasts of the
  normalization scales go through gpsimd partition_broadcast (full fp32).
  ACT: sexp[n] = sum_c exp(S) (free accum), htile = exp((ALPHA/TAU)*S)
  DVE: hout = max(iota==T[n], htile)  -> final H tile (label entry == 1.0)
  Exact label term: D_T[n] = sum_f (Xn - muG)^2 * icG from host-gathered
  means[T]/log_vars[T] (fp16 inputs), computed on gpsimd during the main
  loop; loss_n = ln(sexp_n) + TAU*D_T[n].
"""

import os
import sys

import numpy as np

for _p in ("/opt/trn_rl_repo", "/root/.axon_site/_ro/trn_rl_repo"):
    if _p not in sys.path and os.path.isdir(_p):
        sys.path.insert(0, _p)

N = 16384
C = 2048
F = 128
TAU = 32.0
ALPHA = 0.9
Q = ALPHA / TAU          # scale turning S=-TAU*D into -ALPHA*D
NCORES = 8
NS = N // NCORES         # rows per core (2048)
NT = NS // 128           # 128-row tiles per core (16)

_MODULE_CACHE = {}


def _build_module():
    if "nc" in _MODULE_CACHE:
        return _MODULE_CACHE["nc"]

    import concourse.bacc as bacc
    import concourse.bass as bass
    import concourse.tile as tile
    from concourse import mybir

    dt = mybir.dt
    f32 = dt.float32
    f16 = dt.float16
    R = dt.float16
    i32 = dt.int32
    Alu = mybir.AluOpType
    Act = mybir.ActivationFunctionType

    nc = bacc.Bacc(
        "TRN2",
        target_bir_lowering=False,
        debug=False,
        enable_asserts=False,
        num_devices=NCORES,
    )

    # I/O (per core)
    xt_d = nc.dram_tensor("xt", [F, NS], f32, kind="ExternalInput")       # X.T shard
    mt_d = nc.dram_tensor("mt", [F, C], f32, kind="ExternalInput")        # means.T
    lvt_d = nc.dram_tensor("lvt", [F, C], f32, kind="ExternalInput")      # log_vars.T
    mgt_d = nc.dram_tensor("mgt", [F, NS], f16, kind="ExternalInput")     # means[T].T
    lvgt_d = nc.dram_tensor("lvgt", [F, NS], f16, kind="ExternalInput")   # log_vars[T].T
    tf_d = nc.dram_tensor("tf", [128, NT], f32, kind="ExternalInput")     # labels, [p, t]
    h_d = nc.dram_tensor("h", [NS, C], f32, kind="ExternalOutput")
    loss_d = nc.dram_tensor("loss", [128, 1], f32, kind="ExternalOutput")
    scr_d = nc.dram_tensor("scr", [10, 2048], f32)                        # bounce scratch

    with tile.TileContext(nc) as tc:
        from contextlib import ExitStack

        ctx = ExitStack()
        with ctx:
            singles = ctx.enter_context(tc.tile_pool(name="singles", bufs=1))
            tiny = ctx.enter_context(tc.tile_pool(name="tiny", bufs=1))

            # persistent tiles (live through the main loop); float32r ones are
            # the main-loop matmul operands (DVE rounds on write)
            e_r = singles.tile([128, C], R)        # ic^T
            ast_t = singles.tile([128, C], R)      # 2*TAU*mu*ic (^T)
            st1_t = singles.tile([128, NS], R)     # -TAU*rn^2*X^2 (^T)
            st2_t = singles.tile([128, NS], R)     # Xn^T
            bs_t = singles.tile([1, C], R)         # -TAU*b row
            ones_row_r = tiny.tile([1, 128], R)
            iot_t = singles.tile([128, C], f32)    # iota (class ids along free)
            tf_t = singles.tile([128, NT], f32)
            icg_t = singles.tile([128, NS], f32)   # exp(-clip(log_vars[T]))^T
            mug_t = singles.tile([128, NS], f32)   # l2norm(means)[T]^T
            vch_t = singles.tile([128, NS], f32)   # (Xn - muG)^2 * icG chain
            ones_col = tiny.tile([128, 1], f32)
            ones_row = tiny.tile([1, 128], f32)
            sexp_t = singles.tile([128, NT], f32)

            nc.vector.memset(ones_col[:], 1.0)
            nc.vector.memset(ones_row[:], 1.0)
            nc.vector.tensor_copy(ones_row_r[:], ones_row[:])
            nc.sync.dma_start(out=tf_t[:], in_=tf_d[:, :])

            # iota along classes (f32 exact for values < 2^24; gpsimd
            # iota is in the resident `standard` ucode library)
            nc.gpsimd.iota(
                iot_t[:], pattern=[[1, C]], base=0, channel_multiplier=0,
                allow_small_or_imprecise_dtypes=True,
            )

            # ---------------- prep phase ----------------
            with tc.tile_pool(name="prep", bufs=1) as prep, \
                 tc.tile_pool(name="rows", bufs=2) as rows, \
                 tc.tile_pool(name="prep_psum", bufs=2, space="PSUM") as ppsum:
                xt_t = prep.tile([128, NS], f32)
                mt_t = prep.tile([128, C], f32)
                lvt_t = prep.tile([128, C], f32)
                mgt_t = prep.tile([128, NS], f16)
                lvgt_t = prep.tile([128, NS], f16)
                e_t = prep.tile([128, C], f32)
                xsq_t = prep.tile([128, NS], f32)
                msq_t = prep.tile([128, C], f32)
                mgsq_t = prep.tile([128, NS], f32)
                p1_t = prep.tile([128, C], f32)
                bmat_t = prep.tile([128, C], f32)

                nc.sync.dma_start(out=xt_t[:], in_=xt_d[:, :])
                nc.sync.dma_start(out=mt_t[:], in_=mt_d[:, :])
                nc.sync.dma_start(out=lvt_t[:], in_=lvt_d[:, :])
                nc.sync.dma_start(out=mgt_t[:], in_=mgt_d[:, :])
                nc.sync.dma_start(out=lvgt_t[:], in_=lvgt_d[:, :])

                # ic^T = exp(-clip(lv,0,6)); clip fused in one DVE op (in-place)
                nc.vector.tensor_scalar(
                    out=lvt_t[:], in0=lvt_t[:], scalar1=0.0, scalar2=6.0,
                    op0=Alu.max, op1=Alu.min,
                )
                nc.scalar.activation(out=e_t[:], in_=lvt_t[:], func=Act.Exp, scale=-1.0)
                nc.vector.tensor_copy(e_r[:], e_t[:])  # rounded copy for the PE
                # gathered icG similarly (fp16 in, f32 out)
                nc.vector.tensor_scalar(
                    out=lvgt_t[:], in0=lvgt_t[:], scalar1=0.0, scalar2=6.0,
                    op0=Alu.max, op1=Alu.min,
                )
                nc.scalar.activation(out=icg_t[:], in_=lvgt_t[:], func=Act.Exp, scale=-1.0)

                # squares (gpsimd to keep DVE free)
                nc.gpsimd.tensor_mul(xsq_t[:], xt_t[:], xt_t[:])
                nc.gpsimd.tensor_mul(msq_t[:], mt_t[:], mt_t[:])
                nc.gpsimd.tensor_mul(mgsq_t[:], mgt_t[:], mgt_t[:])

                # column norms^2 via plain-fp32 ones-matmul -> [1,2048] row in
                # PSUM -> SBUF row -> DRAM (re-read as [128,16], n = 16p+j)
                def colsum_to_row(src_t, scr_row):
                    ps = ppsum.tile([128, 2048], f32, tag="ppsum")
                    for ch in range(4):
                        cs = slice(ch * 512, (ch + 1) * 512)
                        nc.tensor.matmul(
                            out=ps[0:1, cs], lhsT=ones_col[:], rhs=src_t[:, cs],
                            start=True, stop=True,
                        )
                    row = rows.tile([1, 2048], f32, tag="rows")
                    nc.vector.tensor_copy(row[:], ps[0:1, :])
                    nc.sync.dma_start(out=scr_d[scr_row : scr_row + 1, :], in_=row[:])

                colsum_to_row(xsq_t, 0)   # ||x||^2 per n
                colsum_to_row(msq_t, 1)   # ||mu||^2 per c
                colsum_to_row(mgsq_t, 2)  # ||means[T]||^2 per n

                def read_pt(scr_row):
                    t = tiny.tile([128, 16], f32)
                    nc.sync.dma_start(
                        out=t[:], in_=scr_d[scr_row, :].rearrange("(p j) -> p j", p=128)
                    )
                    return t

                nx2_t = read_pt(0)
                nm2_t = read_pt(1)
                ng2_t = read_pt(2)

                # rsqrt = exp(-0.5*ln(x)) (keeps everything in the exp/ln table
                # set); guard like the reference's max(norm, 1e-12)
                def rsqrt_pt(src):
                    nc.vector.tensor_scalar(
                        out=src[:], in0=src[:], scalar1=1e-24, scalar2=None, op0=Alu.max
                    )
                    ln = tiny.tile([128, 16], f32)
                    nc.scalar.activation(out=ln[:], in_=src[:], func=Act.Ln)
                    rs = tiny.tile([128, 16], f32)
                    nc.scalar.activation(out=rs[:], in_=ln[:], func=Act.Exp, scale=-0.5)
                    return rs

                rsx_t = rsqrt_pt(nx2_t)   # 1/||x||
                rsm_t = rsqrt_pt(nm2_t)   # 1/||mu||
                rsg_t = rsqrt_pt(ng2_t)   # 1/||means[T]||

                # derived per-column scale rows
                r2x_t = tiny.tile([128, 16], f32)   # -TAU * rn^2
                r3m_t = tiny.tile([128, 16], f32)   # 2*TAU*rsm
                r4m_t = tiny.tile([128, 16], f32)   # -TAU*rsm^2
                nc.vector.tensor_mul(r2x_t[:], rsx_t[:], rsx_t[:])
                nc.vector.tensor_scalar(
                    out=r2x_t[:], in0=r2x_t[:], scalar1=-TAU, scalar2=None, op0=Alu.mult
                )
                nc.vector.tensor_scalar(
                    out=r3m_t[:], in0=rsm_t[:], scalar1=2.0 * TAU, scalar2=None, op0=Alu.mult
                )
                nc.vector.tensor_mul(r4m_t[:], rsm_t[:], rsm_t[:])
                nc.vector.tensor_scalar(
                    out=r4m_t[:], in0=r4m_t[:], scalar1=-TAU, scalar2=None, op0=Alu.mult
                )

                def write_pt(src, scr_row):
                    nc.sync.dma_start(
                        out=scr_d[scr_row, :].rearrange("(p j) -> p j", p=128), in_=src[:]
                    )

                write_pt(rsx_t, 3)
                write_pt(r2x_t, 4)
                write_pt(r3m_t, 5)
                write_pt(r4m_t, 6)
                write_pt(rsg_t, 7)

                # broadcast a scratch row to [128, 2048] via K=1 plain-fp32
                # matmul into PSUM (ones_row (x) row); DVE consumes the PSUM
                def bcast_row(scr_row):
                    row = rows.tile([1, 2048], f32, tag="rows")
                    nc.sync.dma_start(out=row[:], in_=scr_d[scr_row : scr_row + 1, :])
                    b = ppsum.tile([128, 2048], f32, tag="ppsum")
                    for ch in range(4):
                        cs = slice(ch * 512, (ch + 1) * 512)
                        nc.tensor.matmul(
                            out=b[:, cs], lhsT=ones_row[:], rhs=row[:, cs],
                            start=True, stop=True,
                        )
                    return b

                r2xB = bcast_row(4)
                nc.vector.tensor_mul(st1_t[:], xsq_t[:], r2xB[:])   # -TAU*rn^2*X^2
                rxB = bcast_row(3)
                nc.vector.tensor_mul(st2_t[:], xt_t[:], rxB[:])     # Xn^T
                nc.gpsimd.tensor_mul(p1_t[:], mt_t[:], e_t[:])
                r3mB = bcast_row(5)
                nc.vector.tensor_mul(ast_t[:], p1_t[:], r3mB[:])    # 2*TAU*mu*ic
                nc.gpsimd.tensor_mul(msq_t[:], msq_t[:], e_t[:])
                r4mB = bcast_row(6)
                nc.vector.tensor_mul(bmat_t[:], msq_t[:], r4mB[:])  # -TAU*mu^2*ic
                rsgB = bcast_row(7)
                nc.vector.tensor_mul(mug_t[:], mgt_t[:], rsgB[:])   # muG^T

                # bias row: bs[c] = colsum(bmat), evacuated as float32r
                bs_ps = ppsum.tile([128, C], f32, tag="ppsum")
                for ch in range(4):
                    cs = slice(ch * 512, (ch + 1) * 512)
                    nc.tensor.matmul(
                        out=bs_ps[0:1, cs], lhsT=ones_col[:], rhs=bmat_t[:, cs],
                        start=True, stop=True,
                    )
                nc.vector.tensor_copy(bs_t[:], bs_ps[0:1, :])

            # D_T chain on gpsimd — overlaps the main loop below
            nc.gpsimd.tensor_sub(vch_t[:], st2_t[:], mug_t[:])
            nc.gpsimd.tensor_mul(vch_t[:], vch_t[:], vch_t[:])
            nc.gpsimd.tensor_mul(vch_t[:], vch_t[:], icg_t[:])

            # ---------------- main loop ----------------
            with tc.tile_pool(name="spsum", bufs=2, space="PSUM") as spsum, \
                 tc.tile_pool(name="hpool", bufs=3) as hpool, \
                 tc.tile_pool(name="hopool", bufs=3) as hopool, \
                 tc.tile_pool(name="pscr", bufs=1) as pscrp:
                pscr_t = pscrp.tile([128, C], f32)  # exp(S) full-size sink
                for t in range(NT):
                    ns = slice(t * 128, (t + 1) * 128)
                    s_ps = spsum.tile([128, C], f32, tag="S")
                    for ch in range(4):
                        cs = slice(ch * 512, (ch + 1) * 512)
                        nc.tensor.matmul(
                            out=s_ps[:, cs], lhsT=st1_t[:, ns], rhs=e_r[:, cs],
                            start=True, stop=False,
                        )
                        nc.tensor.matmul(
                            out=s_ps[:, cs], lhsT=st2_t[:, ns], rhs=ast_t[:, cs],
                            start=False, stop=False,
                        )
                        nc.tensor.matmul(
                            out=s_ps[:, cs], lhsT=ones_row_r[:], rhs=bs_t[:, cs],
                            start=False, stop=True,
                        )
                    # softmax sum (accum is free on the ACT pass)
                    nc.scalar.activation(
                        out=pscr_t[:], in_=s_ps[:], func=Act.Exp,
                        accum_out=sexp_t[:, t : t + 1],
                    )
                    # H tile = exp((ALPHA/TAU)*S)
                    ht = hpool.tile([128, C], f32, tag="H")
                    nc.scalar.activation(out=ht[:], in_=s_ps[:], func=Act.Exp, scale=Q)
                    # blend one-hot: hout = max(iota == T[n], htile)
                    ho = hopool.tile([128, C], f32, tag="HO")
                    nc.vector.scalar_tensor_tensor(
                        out=ho[:], in0=iot_t[:], scalar=tf_t[:, t : t + 1],
                        in1=ht[:], op0=Alu.is_equal, op1=Alu.max,
                    )
                    nc.sync.dma_start(out=h_d[ns, :], in_=ho[:])

                # D_T[n] = colsum(vch) — reuses a freed S slot at pipeline end
                dt_ps = spsum.tile([128, NS], f32, tag="S")
                for ch in range(4):
                    cs = slice(ch * 512, (ch + 1) * 512)
                    nc.tensor.matmul(
                        out=dt_ps[0:1, cs], lhsT=ones_col[:], rhs=vch_t[:, cs],
                        start=True, stop=True,
                    )
                dt_row = singles.tile([1, NS], f32)
                nc.vector.tensor_copy(dt_row[:], dt_ps[0:1, :])
                nc.sync.dma_start(out=scr_d[8:9, :], in_=dt_row[:])

            # ---------------- loss epilogue ----------------
            dt_t = tiny.tile([128, NT], f32)
            nc.sync.dma_start(
                out=dt_t[:], in_=scr_d[8, :].rearrange("(p j) -> p j", p=128)
            )
            lns_t = tiny.tile([128, NT], f32)
            lossv_t = tiny.tile([128, NT], f32)
            loss_t = tiny.tile([128, 1], f32)
            nc.scalar.activation(out=lns_t[:], in_=sexp_t[:], func=Act.Ln)
            # loss_n = lns + TAU * D_T[n]  (partial sums in arbitrary order;
            # the host only uses the total)
            nc.vector.scalar_tensor_tensor(
                out=lossv_t[:], in0=dt_t[:], scalar=TAU, in1=lns_t[:],
                op0=Alu.mult, op1=Alu.add,
            )
            nc.vector.reduce_sum(
                out=loss_t[:], in_=lossv_t[:], axis=mybir.AxisListType.X
            )
            nc.sync.dma_start(out=loss_d[:, :], in_=loss_t[:])

    nc.compile()
    _MODULE_CACHE["nc"] = nc
    return nc


def _make_in_maps(X, T, means, log_vars):
    X = np.asarray(X, dtype=np.float32)
    T = np.asarray(T).astype(np.int64)
    means = np.asarray(means, dtype=np.float32)
    log_vars = np.asarray(log_vars, dtype=np.float32)

    xt_full = np.ascontiguousarray(X.T)                     # [F, N]
    mt = np.ascontiguousarray(means.T)                      # [F, C]
    lvt = np.ascontiguousarray(log_vars.T)                  # [F, C]
    mg_full = means[T].T.astype(np.float16)                 # [F, N]
    lvg_full = log_vars[T].T.astype(np.float16)             # [F, N]

    in_maps = []
    for c in range(NCORES):
        sl = slice(c * NS, (c + 1) * NS)
        tf = np.ascontiguousarray(
            T[sl].astype(np.float32).reshape(NT, 128).T
        )                                                    # [128, NT]
        in_maps.append(
            {
                "xt": np.ascontiguousarray(xt_full[:, sl]),
                "mt": mt,
                "lvt": lvt,
                "mgt": np.ascontiguousarray(mg_full[:, sl]),
                "lvgt": np.ascontiguousarray(lvg_full[:, sl]),
                "tf": tf,
            }
        )
    return in_maps


def _postprocess(results):
    h_parts = [results[c]["h"] for c in range(NCORES)]
    H = np.concatenate(h_parts, axis=0)
    loss_sum = 0.0
    for c in range(NCORES):
        loss_sum += float(np.sum(results[c]["loss"].astype(np.float64)))
    loss_mean = np.float32(loss_sum / N)
    return loss_mean, H


def kernel(X, T, means, log_vars):
    from concourse.bass_utils import run_bass_kernel_spmd

    nc = _build_module()
    in_maps = _make_in_maps(X, T, means, log_vars)
    res = run_bass_kernel_spmd(nc, in_maps, list(range(NCORES)))
    return _postprocess(res.results)


def run_sim(X, T, means, log_vars, core=0):
    """CoreSim single-core run for correctness debugging (no hardware)."""
    from concourse.bass_interp import CoreSim

    nc = _build_module()
    in_maps = _make_in_maps(X, T, means, log_vars)
    sim = CoreSim(nc, trace=False)
    for k, v in in_maps[core].items():
        sim.tensor(k)[:] = v
    sim.simulate()
    return {k: np.array(sim.tensor(k)) for k in ("h", "loss")}


# revision 9
# speedup vs baseline: 1.5730x; 1.0064x over previous
"""Trainium2 Bass kernel for the Mahalanobis-softmax loss + H matrix.

Computes, for X[16384,128], T[16384], means[2048,128], log_vars[2048,128]:
  lv   = clip(log_vars, 0, 6);  ic = exp(-lv)
  Xn   = l2norm(X); mu = l2norm(means)
  D    = ||Xn - mu||^2_ic  (Mahalanobis, expanded to GEMMs)
  loss = mean_n( -log_softmax(-TAU*D)[n, T[n]] )
  H    = onehot(T) + exp(-ALPHA*D) * (1 - onehot(T))

Sharding: data-parallel over N across 8 NeuronCores (X/T sharded,
means/log_vars replicated); H shards concatenated and per-core partial
loss sums reduced on host.

Device-side formulation (per core, S := -TAU*D computed straight in PSUM):
  S[n,c] = st1(:,n).e(:,c) + st2(:,n).ast(:,c) + bs(c)      (3 matmuls/chunk)
    st1[f,n] = -TAU * rn[n]^2 * X[n,f]^2   (stationary 1)
    st2[f,n] = rn[n] * X[n,f] = Xn^T       (stationary 2)
    e[f,c]   = exp(-clip(lv,0,6))[c,f] = ic^T     (moving 1)
    ast[f,c] = 2*TAU*mu[c,f]*ic[c,f]              (moving 2)
    bs[c]    = -TAU * sum_f mu^2*ic               (K=1 bias row)
  Main-loop matmul operands are float32r (1 cyc/row vs 4 for float32);
  they are all produced by DVE ops which round on write. Small prep
  colsums stay plain fp32 matmuls. Row->tile broadcasts of the
  normalization scales go through gpsimd partition_broadcast (full fp32).
  ACT: sexp[n] = sum_c exp(S) (free accum), htile = exp((ALPHA/TAU)*S)
  DVE: hout = max(iota==T[n], htile)  -> final H tile (label entry == 1.0)
  Exact label term: D_T[n] = sum_f (Xn - muG)^2 * icG from host-gathered
  means[T]/log_vars[T] (fp16 inputs), computed on gpsimd during the main
  loop; loss_n = ln(sexp_n) + TAU*D_T[n].
"""

import os
import sys

import numpy as np

for _p in ("/opt/trn_rl_repo", "/root/.axon_site/_ro/trn_rl_repo"):
    if _p not in sys.path and os.path.isdir(_p):
        sys.path.insert(0, _p)

N = 16384
C = 2048
F = 128
TAU = 32.0
ALPHA = 0.9
Q = ALPHA / TAU          # scale turning S=-TAU*D into -ALPHA*D
SHIFT = 40.0             # exp-input bias keeping ACT spline in its accurate range
NCORES = 8
NS = N // NCORES         # rows per core (2048)
NT = NS // 128           # 128-row tiles per core (16)

_MODULE_CACHE = {}


def _build_module():
    if "nc" in _MODULE_CACHE:
        return _MODULE_CACHE["nc"]

    import concourse.bacc as bacc
    import concourse.bass as bass
    import concourse.tile as tile
    from concourse import mybir

    dt = mybir.dt
    f32 = dt.float32
    f16 = dt.float16
    R = dt.float16
    i32 = dt.int32
    Alu = mybir.AluOpType
    Act = mybir.ActivationFunctionType

    nc = bacc.Bacc(
        "TRN2",
        target_bir_lowering=False,
        debug=False,
        enable_asserts=False,
        num_devices=NCORES,
    )

    # I/O (per core)
    xt_d = nc.dram_tensor("xt", [F, NS], f32, kind="ExternalInput")       # X.T shard
    mt_d = nc.dram_tensor("mt", [F, C], f32, kind="ExternalInput")        # means.T
    lvt_d = nc.dram_tensor("lvt", [F, C], f32, kind="ExternalInput")      # log_vars.T
    mgt_d = nc.dram_tensor("mgt", [F, NS], f16, kind="ExternalInput")     # means[T].T
    lvgt_d = nc.dram_tensor("lvgt", [F, NS], f16, kind="ExternalInput")   # log_vars[T].T
    tf_d = nc.dram_tensor("tf", [128, NT], f32, kind="ExternalInput")     # labels, [p, t]
    h_d = nc.dram_tensor("h", [NS, C], f32, kind="ExternalOutput")
    loss_d = nc.dram_tensor("loss", [128, 1], f32, kind="ExternalOutput")
    scr_d = nc.dram_tensor("scr", [10, 2048], f32)                        # bounce scratch
    scrh_d = nc.dram_tensor("scrh", [8, 2048], f16)                       # fp16 scale rows

    with tile.TileContext(nc) as tc:
        from contextlib import ExitStack

        ctx = ExitStack()
        with ctx:
            singles = ctx.enter_context(tc.tile_pool(name="singles", bufs=1))
            tiny = ctx.enter_context(tc.tile_pool(name="tiny", bufs=1))

            # persistent tiles (live through the main loop); float32r ones are
            # the main-loop matmul operands (DVE rounds on write)
            e_r = singles.tile([128, C], R)        # ic^T
            ast_t = singles.tile([128, C], R)      # 2*TAU*mu*ic (^T)
            st1_t = singles.tile([128, NS], R)     # -TAU*rn^2*X^2 (^T)
            st2_t = singles.tile([128, NS], R)     # Xn^T
            bs_t = singles.tile([1, C], R)         # -TAU*b row
            ones_row_r = tiny.tile([1, 128], R)
            iot_t = singles.tile([128, C], f32)    # iota (class ids along free)
            tf_t = singles.tile([128, NT], f32)
            icg_t = singles.tile([128, NS], f32)   # exp(-clip(log_vars[T]))^T
            mug_t = singles.tile([128, NS], f32)   # l2norm(means)[T]^T
            vch_t = singles.tile([128, NS], f32)   # (Xn - muG)^2 * icG chain
            vchh_t = singles.tile([128, NS], R)    # fp16 twin for the dT colsum
            ones_col = tiny.tile([128, 1], f32)
            ones_col_h = tiny.tile([128, 1], R)
            ones_row = tiny.tile([1, 128], f32)
            sexp_t = singles.tile([128, NT], f32)

            shift_t = tiny.tile([128, 1], f32)
            nc.vector.memset(shift_t[:], SHIFT)
            nc.vector.memset(ones_col[:], 1.0)
            nc.vector.tensor_copy(ones_col_h[:], ones_col[:])
            nc.vector.memset(ones_row[:], 1.0)
            nc.vector.tensor_copy(ones_row_r[:], ones_row[:])
            nc.sync.dma_start(out=tf_t[:], in_=tf_d[:, :])

            # iota along classes (f32 exact for values < 2^24; gpsimd
            # iota is in the resident `standard` ucode library)
            nc.gpsimd.iota(
                iot_t[:], pattern=[[1, C]], base=0, channel_multiplier=0,
                allow_small_or_imprecise_dtypes=True,
            )

            # ---------------- prep phase ----------------
            with tc.tile_pool(name="prep", bufs=1) as prep, \
                 tc.tile_pool(name="rows", bufs=2) as rows, \
                 tc.tile_pool(name="prep_psum", bufs=2, space="PSUM") as ppsum:
                xt_t = prep.tile([128, NS], f32)
                mt_t = prep.tile([128, C], f32)
                lvt_t = prep.tile([128, C], f32)
                mgt_t = prep.tile([128, NS], f16)
                lvgt_t = prep.tile([128, NS], f16)
                xsq_t = prep.tile([128, NS], R)
                msq_t = prep.tile([128, C], R)
                mgsq_t = prep.tile([128, NS], R)
                p1_t = prep.tile([128, C], f32)
                bmat_t = prep.tile([128, C], R)

                nc.sync.dma_start(out=xt_t[:], in_=xt_d[:, :])
                nc.sync.dma_start(out=mt_t[:], in_=mt_d[:, :])
                nc.sync.dma_start(out=lvt_t[:], in_=lvt_d[:, :])
                nc.sync.dma_start(out=mgt_t[:], in_=mgt_d[:, :])
                nc.sync.dma_start(out=lvgt_t[:], in_=lvgt_d[:, :])

                # ic^T = exp(-clip(lv,0,6)); clip fused in one DVE op (in-place)
                nc.vector.tensor_scalar(
                    out=lvt_t[:], in0=lvt_t[:], scalar1=0.0, scalar2=6.0,
                    op0=Alu.max, op1=Alu.min,
                )
                nc.scalar.activation(out=e_r[:], in_=lvt_t[:], func=Act.Exp, scale=-1.0)
                # gathered icG similarly (fp16 in, f32 out)
                nc.vector.tensor_scalar(
                    out=lvgt_t[:], in0=lvgt_t[:], scalar1=0.0, scalar2=6.0,
                    op0=Alu.max, op1=Alu.min,
                )
                nc.scalar.activation(out=icg_t[:], in_=lvgt_t[:], func=Act.Exp, scale=-1.0)

                # squares (gpsimd to keep DVE free)
                nc.gpsimd.tensor_mul(xsq_t[:], xt_t[:], xt_t[:])
                nc.gpsimd.tensor_mul(msq_t[:], mt_t[:], mt_t[:])
                nc.gpsimd.tensor_mul(mgsq_t[:], mgt_t[:], mgt_t[:])

                # column norms^2 via plain-fp32 ones-matmul -> [1,2048] row in
                # PSUM -> SBUF row -> DRAM (re-read as [128,16], n = 16p+j)
                def colsum_to_row(src_t, scr_row):
                    ps = ppsum.tile([128, 2048], f32, tag="ppsum")
                    for ch in range(4):
                        cs = slice(ch * 512, (ch + 1) * 512)
                        nc.tensor.matmul(
                            out=ps[0:1, cs], lhsT=ones_col_h[:], rhs=src_t[:, cs],
                            start=True, stop=True,
                        )
                    row = rows.tile([1, 2048], f32, tag="rows")
                    nc.vector.tensor_copy(row[:], ps[0:1, :])
                    nc.sync.dma_start(out=scr_d[scr_row : scr_row + 1, :], in_=row[:])

                colsum_to_row(xsq_t, 0)   # ||x||^2 per n
                colsum_to_row(msq_t, 1)   # ||mu||^2 per c
                colsum_to_row(mgsq_t, 2)  # ||means[T]||^2 per n

                def read_pt(scr_row):
                    t = tiny.tile([128, 16], f32)
                    nc.sync.dma_start(
                        out=t[:], in_=scr_d[scr_row, :].rearrange("(p j) -> p j", p=128)
                    )
                    return t

                nx2_t = read_pt(0)
                nm2_t = read_pt(1)
                ng2_t = read_pt(2)

                # rsqrt = exp(-0.5*ln(x)) (keeps everything in the exp/ln table
                # set); guard like the reference's max(norm, 1e-12)
                def rsqrt_pt(src):
                    nc.vector.tensor_scalar(
                        out=src[:], in0=src[:], scalar1=1e-24, scalar2=None, op0=Alu.max
                    )
                    ln = tiny.tile([128, 16], f32)
                    nc.scalar.activation(out=ln[:], in_=src[:], func=Act.Ln)
                    rs = tiny.tile([128, 16], f32)
                    nc.scalar.activation(out=rs[:], in_=ln[:], func=Act.Exp, scale=-0.5)
                    return rs

                rsx_t = rsqrt_pt(nx2_t)   # 1/||x||
                rsm_t = rsqrt_pt(nm2_t)   # 1/||mu||
                rsg_t = rsqrt_pt(ng2_t)   # 1/||means[T]||

                # derived per-column scale rows
                r2x_t = tiny.tile([128, 16], R)    # -TAU * rn^2
                r3m_t = tiny.tile([128, 16], R)    # 2*TAU*rsm
                r4m_t = tiny.tile([128, 16], R)    # -TAU*rsm^2
                rsxh_t = tiny.tile([128, 16], R)
                rsgh_t = tiny.tile([128, 16], R)
                nc.vector.tensor_copy(rsxh_t[:], rsx_t[:])
                nc.vector.tensor_copy(rsgh_t[:], rsg_t[:])
                nc.vector.tensor_mul(r2x_t[:], rsx_t[:], rsx_t[:])
                nc.vector.tensor_scalar(
                    out=r2x_t[:], in0=r2x_t[:], scalar1=-TAU, scalar2=None, op0=Alu.mult
                )
                nc.vector.tensor_scalar(
                    out=r3m_t[:], in0=rsm_t[:], scalar1=2.0 * TAU, scalar2=None, op0=Alu.mult
                )
                nc.vector.tensor_mul(r4m_t[:], rsm_t[:], rsm_t[:])
                nc.vector.tensor_scalar(
                    out=r4m_t[:], in0=r4m_t[:], scalar1=-TAU, scalar2=None, op0=Alu.mult
                )

                def write_pt(src, scr_row):
                    nc.sync.dma_start(
                        out=scrh_d[scr_row, :].rearrange("(p j) -> p j", p=128), in_=src[:]
                    )

                write_pt(rsxh_t, 3)
                write_pt(r2x_t, 4)
                write_pt(r3m_t, 5)
                write_pt(r4m_t, 6)
                write_pt(rsgh_t, 7)

                # broadcast a scratch row to [128, 2048] via K=1 plain-fp32
                # matmul into PSUM (ones_row (x) row); DVE consumes the PSUM
                def bcast_row(scr_row):
                    row = rows.tile([1, 2048], R, tag="rows")
                    nc.sync.dma_start(out=row[:], in_=scrh_d[scr_row : scr_row + 1, :])
                    b = ppsum.tile([128, 2048], f32, tag="ppsum")
                    for ch in range(4):
                        cs = slice(ch * 512, (ch + 1) * 512)
                        nc.tensor.matmul(
                            out=b[:, cs], lhsT=ones_row_r[:], rhs=row[:, cs],
                            start=True, stop=True,
                        )
                    return b

                r2xB = bcast_row(4)
                nc.vector.tensor_mul(st1_t[:], xsq_t[:], r2xB[:])   # -TAU*rn^2*X^2
                rxB = bcast_row(3)
                nc.vector.tensor_mul(st2_t[:], xt_t[:], rxB[:])     # Xn^T
                nc.gpsimd.tensor_mul(p1_t[:], mt_t[:], e_r[:])
                r3mB = bcast_row(5)
                nc.vector.tensor_mul(ast_t[:], p1_t[:], r3mB[:])    # 2*TAU*mu*ic
                nc.gpsimd.tensor_mul(msq_t[:], msq_t[:], e_r[:])
                r4mB = bcast_row(6)
                nc.vector.tensor_mul(bmat_t[:], msq_t[:], r4mB[:])  # -TAU*mu^2*ic
                rsgB = bcast_row(7)
                nc.vector.tensor_mul(mug_t[:], mgt_t[:], rsgB[:])   # muG^T

                # bias row: bs[c] = colsum(bmat), evacuated as float32r
                bs_ps = ppsum.tile([128, C], f32, tag="ppsum")
                for ch in range(4):
                    cs = slice(ch * 512, (ch + 1) * 512)
                    nc.tensor.matmul(
                        out=bs_ps[0:1, cs], lhsT=ones_col_h[:], rhs=bmat_t[:, cs],
                        start=True, stop=True,
                    )
                nc.vector.tensor_copy(bs_t[:], bs_ps[0:1, :])

            # D_T chain on gpsimd — overlaps the main loop below
            nc.gpsimd.tensor_sub(vch_t[:], st2_t[:], mug_t[:])
            nc.gpsimd.tensor_mul(vch_t[:], vch_t[:], vch_t[:])
            nc.gpsimd.tensor_mul(vchh_t[:], vch_t[:], icg_t[:])

            # ---------------- main loop ----------------
            with tc.tile_pool(name="spsum", bufs=2, space="PSUM") as spsum, \
                 tc.tile_pool(name="hpool", bufs=3) as hpool, \
                 tc.tile_pool(name="hopool", bufs=3) as hopool, \
                 tc.tile_pool(name="pscr", bufs=1) as pscrp:
                pscr_t = pscrp.tile([128, C], f32)  # exp(S) full-size sink
                for t in range(NT):
                    ns = slice(t * 128, (t + 1) * 128)
                    s_ps = spsum.tile([128, C], f32, tag="S")
                    for ch in range(4):
                        cs = slice(ch * 512, (ch + 1) * 512)
                        nc.tensor.matmul(
                            out=s_ps[:, cs], lhsT=st1_t[:, ns], rhs=e_r[:, cs],
                            start=True, stop=False,
                        )
                        nc.tensor.matmul(
                            out=s_ps[:, cs], lhsT=st2_t[:, ns], rhs=ast_t[:, cs],
                            start=False, stop=False,
                        )
                        nc.tensor.matmul(
                            out=s_ps[:, cs], lhsT=ones_row_r[:], rhs=bs_t[:, cs],
                            start=False, stop=True,
                        )
                    # softmax sum (accum is free on the ACT pass)
                    nc.scalar.activation(
                        out=pscr_t[:], in_=s_ps[:], func=Act.Exp, bias=shift_t[:],
                        accum_out=sexp_t[:, t : t + 1],
                    )
                    # H tile = exp((ALPHA/TAU)*S)
                    ht = hpool.tile([128, C], f32, tag="H")
                    nc.scalar.activation(out=ht[:], in_=s_ps[:], func=Act.Exp, scale=Q)
                    # blend one-hot: hout = max(iota == T[n], htile)
                    ho = hopool.tile([128, C], f32, tag="HO")
                    nc.vector.scalar_tensor_tensor(
                        out=ho[:], in0=iot_t[:], scalar=tf_t[:, t : t + 1],
                        in1=ht[:], op0=Alu.is_equal, op1=Alu.max,
                    )
                    nc.sync.dma_start(out=h_d[ns, :], in_=ho[:])

                # D_T[n] = colsum(vch) — reuses a freed S slot at pipeline end
                dt_ps = spsum.tile([128, NS], f32, tag="S")
                for ch in range(4):
                    cs = slice(ch * 512, (ch + 1) * 512)
                    nc.tensor.matmul(
                        out=dt_ps[0:1, cs], lhsT=ones_col_h[:], rhs=vchh_t[:, cs],
                        start=True, stop=True,
                    )
                dt_row = singles.tile([1, NS], f32)
                nc.vector.tensor_copy(dt_row[:], dt_ps[0:1, :])
                nc.sync.dma_start(out=scr_d[8:9, :], in_=dt_row[:])

            # ---------------- loss epilogue ----------------
            dt_t = tiny.tile([128, NT], f32)
            nc.sync.dma_start(
                out=dt_t[:], in_=scr_d[8, :].rearrange("(p j) -> p j", p=128)
            )
            lns_t = tiny.tile([128, NT], f32)
            lossv_t = tiny.tile([128, NT], f32)
            loss_t = tiny.tile([128, 1], f32)
            nc.scalar.activation(out=lns_t[:], in_=sexp_t[:], func=Act.Ln)
            # loss_n = lns + TAU * D_T[n]  (partial sums in arbitrary order;
            # the host only uses the total)
            nc.vector.scalar_tensor_tensor(
                out=lossv_t[:], in0=dt_t[:], scalar=TAU, in1=lns_t[:],
                op0=Alu.mult, op1=Alu.add,
            )
            nc.vector.reduce_sum(
                out=loss_t[:], in_=lossv_t[:], axis=mybir.AxisListType.X
            )
            nc.sync.dma_start(out=loss_d[:, :], in_=loss_t[:])

    nc.compile()
    _MODULE_CACHE["nc"] = nc
    return nc


def _make_in_maps(X, T, means, log_vars):
    X = np.asarray(X, dtype=np.float32)
    T = np.asarray(T).astype(np.int64)
    means = np.asarray(means, dtype=np.float32)
    log_vars = np.asarray(log_vars, dtype=np.float32)

    xt_full = np.ascontiguousarray(X.T)                     # [F, N]
    mt = np.ascontiguousarray(means.T)                      # [F, C]
    lvt = np.ascontiguousarray(log_vars.T)                  # [F, C]
    mg_full = means[T].T.astype(np.float16)                 # [F, N]
    lvg_full = log_vars[T].T.astype(np.float16)             # [F, N]

    in_maps = []
    for c in range(NCORES):
        sl = slice(c * NS, (c + 1) * NS)
        tf = np.ascontiguousarray(
            T[sl].astype(np.float32).reshape(NT, 128).T
        )                                                    # [128, NT]
        in_maps.append(
            {
                "xt": np.ascontiguousarray(xt_full[:, sl]),
                "mt": mt,
                "lvt": lvt,
                "mgt": np.ascontiguousarray(mg_full[:, sl]),
                "lvgt": np.ascontiguousarray(lvg_full[:, sl]),
                "tf": tf,
            }
        )
    return in_maps


def _postprocess(results):
    h_parts = [results[c]["h"] for c in range(NCORES)]
    H = np.concatenate(h_parts, axis=0)
    loss_sum = 0.0
    for c in range(NCORES):
        loss_sum += float(np.sum(results[c]["loss"].astype(np.float64)))
    loss_mean = np.float32(loss_sum / N - SHIFT)
    return loss_mean, H


def kernel(X, T, means, log_vars):
    from concourse.bass_utils import run_bass_kernel_spmd

    nc = _build_module()
    in_maps = _make_in_maps(X, T, means, log_vars)
    res = run_bass_kernel_spmd(nc, in_maps, list(range(NCORES)))
    return _postprocess(res.results)


def run_sim(X, T, means, log_vars, core=0):
    """CoreSim single-core run for correctness debugging (no hardware)."""
    from concourse.bass_interp import CoreSim

    nc = _build_module()
    in_maps = _make_in_maps(X, T, means, log_vars)
    sim = CoreSim(nc, trace=False)
    for k, v in in_maps[core].items():
        sim.tensor(k)[:] = v
    sim.simulate()
    return {k: np.array(sim.tensor(k)) for k in ("h", "loss")}
